# revision 12
# baseline (speedup 1.0000x reference)
"""Trainium2 Bass kernel for nn_BiInteraction (segment softmax bi-interaction).

Strategy (data-parallel over molecules, 8 NeuronCores):
  - Each core owns 8 molecules (its contiguous slice of the batch) and gets
    its slice of protSeq_embed in two layouts (host-transposed protT for the
    score matmuls; natural-layout chunks for the attention pool), its atoms
    padded to 64 slots per molecule (pads are REPLICAS of a real atom, which
    keeps every max reduction exact without masks), an indicator matrix for
    segment sums, and the replicated MLP weights.
  - All matmul operands are fp16 (PSUM accumulation stays fp32): single-pass
    PE matmuls and half the HBM traffic. End-to-end error vs the fp32
    reference is ~1e-3.
  - DMA plan: everything is issued up front, balanced across the two HWDGE
    queues (sync/scalar) plus the slower gpsimd SW queue, ordered so the
    score inputs (protT) land first, the pool inputs (pnat) next, and the
    MLP weights last.  Scalar's stream starts ~1.3us late (act-table load),
    so its first tensors are the ones with the most slack.
  - Scores S[a, l] = (atom @ W_att) . prot[seg(a), l] computed
    block-diagonally: one matmul per molecule, two molecules per PSUM bank.
  - Segment softmax over atoms:   Wc = exp(max_l S); Sc = 1^T (ind * Wc).
  - Residue softmax over protein: Wp = max_a S via PE transpose + grouped
    reduce; one fused exp per stack covers Wc and Wp.
  - Pools via matmuls; normalization via a k=1 broadcast matmul +
    reciprocal; then a single-group 3-layer MLP ([256]->512->256->1 for the
    core's 8 molecules) with one activation per 128-chunk.
  - PSUM->SBUF copies are spread over vector/gpsimd/scalar so no single
    engine serializes the middle of the kernel.

All shapes are static and identical across cores (single SPMD program);
per-core differences (counts, indicators, padding) live in the DMA'd data.
"""

import numpy as np

import concourse.bacc as bacc
import concourse.bass as bass
import concourse.tile as tile
from concourse import mybir
from concourse.bass_utils import run_bass_kernel_spmd

F32 = mybir.dt.float32
F16 = mybir.dt.float16
AxX = mybir.AxisListType.X
AF = mybir.ActivationFunctionType

A, L, D, B = 2048, 512, 128, 64
H1, H2 = 512, 256
NCORES = 8
MPC = B // NCORES            # molecules per core = 8
NPAD = 64                    # padded atom slots per molecule
NSTACK = MPC * NPAD // 128   # stacks of 128 padded atoms per core = 4

# fp16 consts tensor column layout (inside atomw)
C_IDENT = 0        # [0, 128)   identity
C_IND = 128        # [128, 136) indicator, col = molecule
C_ONES = 136       # [136, 137) ones column
C_WO = 137         # [137, 139) Wo chunks
C_W = 139

N_WARM = 10        # PE warm-up matmuls (clock ramp) before real work

_PROGRAM_CACHE = {}


def _build_program():
    nc = bacc.Bacc("TRN2", target_bir_lowering=False, debug=False)

    # atomw = atomT | watt | consts (everything needed early, one DMA)
    AW_W = MPC * NPAD + D + C_W
    d_atomw = nc.dram_tensor("atomw", [128, AW_W], F16, kind="ExternalInput")
    d_protp = [
        nc.dram_tensor(f"protp{q}", [128, 2 * L], F16, kind="ExternalInput")
        for q in range(4)
    ]
    # pnat: natural-layout prot chunks; 0 = mols 0-3, a = mols 4-5, b = 6-7
    d_pnat0 = nc.dram_tensor("pnat0", [128, 4 * L], F16, kind="ExternalInput")
    d_pnata = nc.dram_tensor("pnata", [128, 2 * L], F16, kind="ExternalInput")
    d_pnatb = nc.dram_tensor("pnatb", [128, 2 * L], F16, kind="ExternalInput")
    d_cons2 = nc.dram_tensor("cons2", [128, NSTACK * D], F16, kind="ExternalInput")
    d_w1 = nc.dram_tensor("w1d", [128, 2 * H1], F16, kind="ExternalInput")
    d_w2 = nc.dram_tensor("w2d", [128, 4 * H2], F16, kind="ExternalInput")
    d_bias = nc.dram_tensor("biasc", [128, 6], F32, kind="ExternalInput")
    d_row = nc.dram_tensor("row", [1, 129], F16, kind="ExternalInput")
    d_y = nc.dram_tensor("y", [MPC, 1], F32, kind="ExternalOutput")
    d_warm = nc.dram_tensor("warmo", [1, 1], F32, kind="ExternalOutput")

    with tile.TileContext(nc) as tc:
        with (
            tc.tile_pool(name="weights", bufs=1) as wpool,
            tc.tile_pool(name="work", bufs=1) as work,
            tc.tile_pool(name="spool", bufs=4) as spool,
            tc.tile_pool(name="psum_big", bufs=3, space=bass.MemorySpace.PSUM) as pbig,
            tc.tile_pool(name="psum_q", bufs=3, space=bass.MemorySpace.PSUM) as pq,
            tc.tile_pool(name="psum_s", bufs=2, space=bass.MemorySpace.PSUM) as ps,
        ):
            # ---- loads: all issued immediately, per-queue FIFO ----------
            atomw = wpool.tile([128, AW_W], F16)
            protp = []
            for q in range(4):
                pt = wpool.tile([128, 2 * L], F16, tag=f"protp{q}")
                protp.append(pt)
            pnat0 = wpool.tile([128, 4 * L], F16, tag="pnat0")
            pnata = wpool.tile([128, 2 * L], F16, tag="pnata")
            pnatb = wpool.tile([128, 2 * L], F16, tag="pnatb")
            cons2 = wpool.tile([128, NSTACK * D], F16)
            w1 = wpool.tile([128, 2 * H1], F16, tag="w1t")
            w2 = wpool.tile([128, 4 * H2], F16, tag="w2t")
            biasc = wpool.tile([128, 6], F32)
            row = wpool.tile([1, 129], F16)

            # sync queue: atomw -> protp0 -> protp2 -> pnat0 -> w2
            nc.sync.dma_start(atomw[:], d_atomw[:])
            nc.sync.dma_start(protp[0][:], d_protp[0][:])
            nc.sync.dma_start(protp[2][:], d_protp[2][:])
            nc.sync.dma_start(pnat0[:], d_pnat0[:])
            nc.sync.dma_start(w2[:], d_w2[:])
            # scalar queue (starts late: act-table): protp1 -> protp3 ->
            # pnat45 -> w1
            nc.scalar.dma_start(protp[1][:], d_protp[1][:])
            nc.scalar.dma_start(protp[3][:], d_protp[3][:])
            nc.scalar.dma_start(pnata[:], d_pnata[:])
            nc.scalar.dma_start(w1[:], d_w1[:])
            # gpsimd SW queue (slow): small + late-needed
            nc.gpsimd.dma_start(cons2[:], d_cons2[:])
            nc.gpsimd.dma_start(biasc[:], d_bias[:])
            nc.gpsimd.dma_start(row[:], d_row[:])
            nc.gpsimd.dma_start(pnatb[:], d_pnatb[:])

            atomT = atomw[:, 0 : MPC * NPAD]
            watt = atomw[:, MPC * NPAD : MPC * NPAD + D]
            consts = atomw[:, MPC * NPAD + D :]
            protT = [protp[i // 2][:, (i % 2) * L : (i % 2 + 1) * L] for i in range(MPC)]
            pnat = [pnat0[:, i * L : (i + 1) * L] for i in range(4)]
            pnat += [pnata[:, i * L : (i + 1) * L] for i in range(2)]
            pnat += [pnatb[:, i * L : (i + 1) * L] for i in range(2)]
            atomN = cons2[:].rearrange("p (s d) -> p s d", s=NSTACK)

            ident = consts[:, C_IDENT : C_IDENT + 128]
            ones_col = consts[:, C_ONES : C_ONES + 1]

            # ---- HAM warm-up: ramp the PE clock while DMAs stream ------
            warm = work.tile([128, 256], F16)
            nc.vector.memset(warm[:], 0.0)
            ps_warm = pq.tile([128, 256], F32, tag="q")
            for _ in range(N_WARM):
                nc.tensor.matmul(
                    ps_warm[:], warm[:, :128], warm[:], start=True, stop=True
                )
            warm_out = work.tile([1, 1], F32)
            nc.vector.tensor_copy(warm_out[:], ps_warm[0:1, 0:1])
            nc.gpsimd.dma_start(d_warm[:], warm_out[:])

            # ---- XT = W_att.T-applied atoms: XT[d', a] -----------------
            ps_xt = pbig.tile([128, MPC * NPAD], F32, tag="big")
            nc.tensor.matmul(ps_xt[:], watt[:], atomT[:], start=True, stop=True)
            xt = work.tile([128, MPC * NPAD], F16)
            nc.vector.tensor_copy(xt[:, 0:256], ps_xt[:, 0:256])
            nc.scalar.copy(xt[:, 256:512], ps_xt[:, 256:512])

            # ---- scores: S[a, l] per molecule, stacked 2/psum bank -----
            # PE order interleaves scores and transposes so each stack's
            # S^T is in PSUM as soon as possible after its protT lands.
            s_psums = []
            s_all = work.tile([128, NSTACK * L], F16)
            ps_sts = []
            wpe = work.tile([128, 4 + 8 * NSTACK], F32)
            ewx = work.tile([128, 4 + 8 * NSTACK], F16)

            def score(s):
                ps_S = pbig.tile([128, L], F32, tag="big")
                s_psums.append(ps_S)
                for slot in range(2):
                    i = 2 * s + slot
                    nc.tensor.matmul(
                        ps_S[slot * NPAD : (slot + 1) * NPAD, :],
                        xt[:, i * NPAD : (i + 1) * NPAD],
                        protT[i],
                        start=True,
                        stop=True,
                    )
                # S to SBUF: half on vector, half on scalar
                sb = s_all[:, s * L : (s + 1) * L]
                nc.vector.tensor_copy(sb[:, 0:256], ps_S[:, 0:256])
                nc.scalar.copy(sb[:, 256:512], ps_S[:, 256:512])

            def transp(s):
                sb = s_all[:, s * L : (s + 1) * L]
                ps_st = pq.tile([128, 4 * 128], F16, tag="q")
                ps_sts.append(ps_st)
                for j in range(4):
                    nc.tensor.transpose(
                        ps_st[:, j * 128 : (j + 1) * 128],
                        sb[:, j * 128 : (j + 1) * 128],
                        ident,
                    )

            score(0)
            score(1)
            transp(0)
            score(2)
            transp(1)
            score(3)
            transp(2)
            transp(3)

            # ---- Wp (residue max) per stack on vector; Wc batched ------
            # wpe col layout: col s = Wc(stack s); col 4+8s+2j+g = Wp.
            for s in range(NSTACK):
                nc.vector.reduce_max(
                    wpe[:, 4 + 8 * s : 12 + 8 * s],
                    ps_sts[s][:].rearrange("p (j g k) -> p j g k", j=4, k=NPAD),
                    axis=AxX,
                )
                if s % 2 == 1:
                    nc.scalar.activation(
                        ewx[:, 4 + 8 * (s - 1) : 20 + 8 * (s - 1)],
                        wpe[:, 4 + 8 * (s - 1) : 20 + 8 * (s - 1)],
                        AF.Exp,
                    )
            nc.vector.reduce_max(
                wpe[:, 0:4],
                s_all[:].rearrange("p (s l) -> p s l", s=NSTACK),
                axis=AxX,
            )
            wcf = work.tile([128, 4], F32)
            nc.scalar.activation(wcf[:], wpe[:, 0:4], AF.Exp)

            # t partial sums (per molecule, over the 4 chunks)
            tpart = work.tile([128, MPC], F16)
            with nc.allow_low_precision(reason="sum of 4 fp16 values, 5e-4 rel"):
                nc.vector.reduce_sum(
                    tpart[:].rearrange("p (s g) -> p s g", g=2),
                    ewx[:, 4:].rearrange("p (s j g) -> p s g j", j=4, g=2),
                    axis=AxX,
                )

            # ---- row-form prot pools (packed 4/column-group) -----------
            prows = []
            for g in range(2):
                ps_pr = pq.tile([128, 128], F32, tag="q")
                prows.append(ps_pr)
                for j in range(4):
                    for sl in range(4):
                        m = 4 * g + sl
                        ewc = 4 + 8 * (m // 2) + 2 * j + (m % 2)
                        nc.tensor.matmul(
                            ps_pr[32 * sl : 32 * sl + 1, :],
                            ewx[:, ewc : ewc + 1],
                            pnat[m][:, j * 128 : (j + 1) * 128],
                            start=(j == 0),
                            stop=(j == 3),
                            tile_position=(0, 32 * sl),
                        )
            pr_sb0 = work.tile([128, 128], F16)
            nc.vector.tensor_copy(pr_sb0[:], prows[0][:])
            pr_sb1 = work.tile([128, 128], F16)
            nc.scalar.copy(pr_sb1[:], prows[1][:])
            ps_ppT = []
            for g in range(2):
                ps_pt = pq.tile([128, 128], F16, tag="q")
                nc.tensor.transpose(ps_pt[:], (pr_sb0 if g == 0 else pr_sb1)[:], ident)
                ps_ppT.append(ps_pt)

            # ---- denominators: Sc and t, split so htop never waits t ---
            wcseg = work.tile([128, MPC], F16)
            for s in range(NSTACK):
                nc.gpsimd.tensor_scalar_mul(
                    wcseg[:, 2 * s : 2 * s + 2],
                    in0=consts[:, C_IND + 2 * s : C_IND + 2 * s + 2],
                    scalar1=wcf[:, s : s + 1],
                )
            ps_sc = ps.tile([1, MPC], F32, tag="sp")
            nc.tensor.matmul(ps_sc[:], ones_col, wcseg[:], start=True, stop=True)
            ps_t = ps.tile([1, MPC], F32, tag="sp")
            nc.tensor.matmul(ps_t[:], ones_col, tpart[:], start=True, stop=True)

            sct = work.tile([1, 2 * MPC], F16)
            nc.vector.tensor_copy(sct[:, :MPC], ps_sc[:])
            nc.vector.tensor_copy(sct[:, MPC:], ps_t[:])
            inv = work.tile([128, 2 * MPC], F32)
            ps_bs = ps.tile([128, MPC], F32, tag="sp")
            nc.tensor.matmul(ps_bs[:], row[:, :128], sct[:, :MPC], start=True, stop=True)
            nc.vector.reciprocal(inv[:, :MPC], ps_bs[:])
            ps_bt = ps.tile([128, MPC], F32, tag="sp")
            nc.tensor.matmul(ps_bt[:], row[:, :128], sct[:, MPC:], start=True, stop=True)
            nc.vector.reciprocal(inv[:, MPC:], ps_bt[:])

            # ---- pools -------------------------------------------------
            ps_ap = ps.tile([128, MPC], F32, tag="sp")
            for s in range(NSTACK):
                nc.tensor.matmul(
                    ps_ap[:, 2 * s : 2 * s + 2],
                    atomN[:, s, :],
                    wcseg[:, 2 * s : 2 * s + 2],
                    start=True,
                    stop=True,
                )
            htop = work.tile([128, MPC], F16)
            nc.vector.tensor_mul(htop[:], ps_ap[:], inv[:, :MPC])
            hbot = work.tile([128, MPC], F16)
            for g in range(2):
                nc.vector.tensor_mul(
                    hbot[:, 4 * g : 4 * g + 4],
                    ps_ppT[g][:].rearrange("p (a b) -> p b a", b=32)[:, 0, :],
                    inv[:, MPC + 4 * g : MPC + 4 * g + 4],
                )

            # ---- MLP: single group, one activation per 128-chunk -------
            h1 = work.tile([128, 4 * MPC], F16)
            ps_h1 = ps.tile([128, 4 * MPC], F32, tag="sp")
            for mc in range(4):
                nc.tensor.matmul(
                    ps_h1[:, mc * MPC : (mc + 1) * MPC],
                    w1[:, mc * 128 : (mc + 1) * 128],
                    htop[:],
                    start=True,
                    stop=False,
                )
                nc.tensor.matmul(
                    ps_h1[:, mc * MPC : (mc + 1) * MPC],
                    w1[:, H1 + mc * 128 : H1 + (mc + 1) * 128],
                    hbot[:],
                    start=False,
                    stop=True,
                )
                nc.scalar.activation(
                    h1[:, mc * MPC : (mc + 1) * MPC],
                    ps_h1[:, mc * MPC : (mc + 1) * MPC],
                    AF.Relu,
                    bias=biasc[:, mc : mc + 1],
                )
            h2 = work.tile([128, 2 * MPC], F16)
            ps_h2 = ps.tile([128, 2 * MPC], F32, tag="sp")
            for mc2 in range(2):
                for kc in range(4):
                    nc.tensor.matmul(
                        ps_h2[:, mc2 * MPC : (mc2 + 1) * MPC],
                        w2[:, kc * H2 + mc2 * 128 : kc * H2 + (mc2 + 1) * 128],
                        h1[:, kc * MPC : (kc + 1) * MPC],
                        start=(kc == 0),
                        stop=(kc == 3),
                    )
                nc.scalar.activation(
                    h2[:, mc2 * MPC : (mc2 + 1) * MPC],
                    ps_h2[:, mc2 * MPC : (mc2 + 1) * MPC],
                    AF.Relu,
                    bias=biasc[:, 4 + mc2 : 4 + mc2 + 1],
                )
            ps_o = ps.tile([MPC, 1], F32, tag="sp")
            nc.tensor.matmul(
                ps_o[:], h2[:, :MPC], consts[:, C_WO : C_WO + 1], start=True, stop=False
            )
            nc.tensor.matmul(
                ps_o[:],
                h2[:, MPC : 2 * MPC],
                consts[:, C_WO + 1 : C_WO + 2],
                start=False,
                stop=False,
            )
            nc.tensor.matmul(
                ps_o[:], row[:, :MPC], row[:, 128:129], start=False, stop=True
            )
            y_sb = work.tile([MPC, 1], F32)
            nc.vector.tensor_copy(y_sb[:], ps_o[:])
            nc.sync.dma_start(d_y[:], y_sb[:])

    nc.compile()
    return nc


def _prep_inputs(atom_embed, protSeq_embed, atom_splits, W_att, W1, b1, W2, b2, Wo, bo):
    f16 = np.float16
    atom = np.asarray(atom_embed, dtype=np.float32)
    prot = np.asarray(protSeq_embed, dtype=np.float32)
    splits = np.asarray(atom_splits).astype(np.int64).ravel()
    order = np.argsort(splits, kind="stable")
    counts = np.bincount(splits, minlength=B)
    assert counts.max() <= NPAD, f"molecule with {counts.max()} atoms > NPAD={NPAD}"
    assert counts.min() >= 1, "empty molecule (reference produces NaN there)"
    offs = np.concatenate([[0], np.cumsum(counts)])

    atomP = np.empty((B, NPAD, D), np.float32)
    ind = np.zeros((B, NPAD), np.float32)
    for b in range(B):
        idx = order[offs[b] : offs[b + 1]]
        n = len(idx)
        atomP[b, :n] = atom[idx]
        atomP[b, n:] = atom[idx[0]]  # replicate a real atom: maxes stay exact
        ind[b, :n] = 1.0

    w_att = np.asarray(W_att, np.float32).astype(f16)  # [128, 128]
    w1h = (
        np.asarray(W1, np.float32)
        .reshape(2, 128, H1).transpose(1, 0, 2).reshape(128, 2 * H1).astype(f16)
    )
    w2h = (
        np.asarray(W2, np.float32)
        .reshape(4, 128, H2).transpose(1, 0, 2).reshape(128, 4 * H2).astype(f16)
    )
    b1c = np.asarray(b1, np.float32).reshape(4, 128).T
    b2c = np.asarray(b2, np.float32).reshape(2, 128).T
    biasc = np.zeros((128, 6), np.float32)
    biasc[:, 0:4] = b1c
    biasc[:, 4:6] = b2c
    woc = np.asarray(Wo, np.float32).reshape(2, 128).T.astype(f16)
    row = np.zeros((1, 129), f16)
    row[0, :128] = 1.0
    row[0, 128] = np.asarray(bo, np.float32).ravel()[0]

    in_maps = []
    for c in range(NCORES):
        sl = slice(c * MPC, (c + 1) * MPC)
        protT_c = np.ascontiguousarray(
            prot[sl].transpose(0, 2, 1).astype(f16)
        )  # [MPC, 128, L]
        pnat_c = np.ascontiguousarray(
            prot[sl].reshape(MPC, 4, 128, D).transpose(0, 2, 1, 3)
            .reshape(MPC, 128, L).astype(f16)
        )
        atomT_c = np.ascontiguousarray(atomP[sl].reshape(MPC * NPAD, D).T.astype(f16))
        atomN_c = np.ascontiguousarray(
            atomP[sl].reshape(NSTACK, 128, D).transpose(1, 0, 2)
            .reshape(128, NSTACK * D).astype(f16)
        )
        ind_c = np.zeros((128, MPC), f16)
        for m in range(MPC):
            s, slot = divmod(m, 2)
            ind_c[slot * NPAD : (slot + 1) * NPAD, m] = ind[c * MPC + m]
        consts = np.zeros((128, C_W), f16)
        consts[:, C_IDENT : C_IDENT + 128] = np.eye(128, dtype=f16)
        consts[:, C_IND : C_IND + MPC] = ind_c
        consts[:, C_ONES] = 1.0
        consts[:, C_WO : C_WO + 2] = woc
        im = {
            "atomw": np.ascontiguousarray(
                np.concatenate([atomT_c, w_att, consts], axis=1)
            ),
            "cons2": atomN_c,
            "w1d": w1h,
            "w2d": w2h,
            "biasc": biasc,
            "row": row,
        }
        for q in range(4):
            im[f"protp{q}"] = np.ascontiguousarray(
                protT_c[2 * q : 2 * q + 2].transpose(1, 0, 2).reshape(128, 2 * L)
            )
        im["pnat0"] = np.ascontiguousarray(
            pnat_c[0:4].transpose(1, 0, 2).reshape(128, 4 * L)
        )
        im["pnata"] = np.ascontiguousarray(
            pnat_c[4:6].transpose(1, 0, 2).reshape(128, 2 * L)
        )
        im["pnatb"] = np.ascontiguousarray(
            pnat_c[6:8].transpose(1, 0, 2).reshape(128, 2 * L)
        )
        in_maps.append(im)
    return in_maps


def kernel(atom_embed, protSeq_embed, atom_splits, W_att, W1, b1, W2, b2, Wo, bo,
           _trace=False):
    if "nc" not in _PROGRAM_CACHE:
        _PROGRAM_CACHE["nc"] = _build_program()
    nc = _PROGRAM_CACHE["nc"]
    in_maps = _prep_inputs(
        atom_embed, protSeq_embed, atom_splits, W_att, W1, b1, W2, b2, Wo, bo
    )
    res = run_bass_kernel_spmd(
        nc, in_maps, core_ids=list(range(NCORES)), trace=_trace
    )
    _PROGRAM_CACHE["last_result"] = res
    out = np.concatenate([res.results[c]["y"] for c in range(NCORES)], axis=0)
    return out.astype(np.float32)


# revision 15
# speedup vs baseline: 1.1465x; 1.1465x over previous
"""Trainium2 Bass kernel for nn_BiInteraction (segment softmax bi-interaction).

Strategy (data-parallel over molecules, 8 NeuronCores):
  - Each core owns 8 molecules and gets its slice of protSeq_embed in two
    layouts (host-transposed protT for the score matmuls; natural-layout
    chunks for the attention pool), its atoms padded to 64 slots per
    molecule (pads are REPLICAS of a real atom so max reductions stay
    exact), an indicator matrix for segment sums, and the MLP weights.
  - All matmul operands are fp16 (PSUM stays fp32): single-pass PE matmuls
    and half the HBM traffic. End-to-end error vs fp32 reference ~1e-3.
  - DMA plan: 10 HWDGE loads with sem-allocation-aware call order (first 8
    calls get fresh completion sems; later calls reuse sems whose owner
    completed early, so no issue-time stalls). Sync queue carries the bulk
    ordered by first use; scalar queue is kept light so the Activation
    engine is free for copies/exps early; gpsimd SW queue gets small/late
    tensors. The ones-row + output bias live inside atomw's consts block.
  - Scores S[a, l] per molecule, two molecules per PSUM bank. S is copied
    to SBUF split across vector/scalar, then PE-transposed per 128-chunk.
  - Residue max Wp: vector grouped reduce per stack. Atom max Wc: two
    fp16 max-folds on gpsimd (SBUF only) + a short vector reduce, keeping
    the vector engine mostly free for Wp.
  - exp per stack on scalar as soon as its Wp lands; Wc exp to fp16.
  - Segment sums Sc via tiny PE matmuls of wcf16 against the indicator;
    t via a grouped fp16 sum + ones-matmul; both broadcast to 128
    partitions with a k=1 ones-row matmul, then reciprocals on vector.
  - prot pools: early molecule group uses row-form matmuls packed 4 per PE
    column-group + one transpose; late group uses stationary-pnat matmuls
    that produce columns directly (no transpose on the critical path).
  - Single-group 3-layer MLP ([256]->512->256->1): per-128-chunk PSUM
    tiles (avoids false WAR serialization), relu+bias on scalar.

All shapes are static and identical across cores (single SPMD program);
per-core differences (counts, indicators, padding) live in the DMA'd data.
"""

import numpy as np

import concourse.bacc as bacc
import concourse.bass as bass
import concourse.tile as tile
from concourse import mybir
from concourse.bass_utils import run_bass_kernel_spmd

F32 = mybir.dt.float32
F16 = mybir.dt.float16
AxX = mybir.AxisListType.X
AF = mybir.ActivationFunctionType

A, L, D, B = 2048, 512, 128, 64
H1, H2 = 512, 256
NCORES = 8
MPC = B // NCORES            # molecules per core = 8
NPAD = 64                    # padded atom slots per molecule
NSTACK = MPC * NPAD // 128   # stacks of 128 padded atoms per core = 4

# fp16 consts tensor column layout (inside atomw)
C_IDENT = 0          # [0, 128)   identity
C_IND = 128          # [128, 136) indicator, col = molecule
C_ONES = 136         # [136, 137) ones column
C_WO = 137           # [137, 139) Wo chunks
C_ROW = 139          # [139, 267) row 0 = ones; col 267 row 0 = bo
C_W = 268

N_WARM = 10          # PE warm-up matmuls before XT
N_FILL = 2           # extra warm matmuls after XT (keep clock ramping)

_PROGRAM_CACHE = {}


def _build_program():
    nc = bacc.Bacc("TRN2", target_bir_lowering=False, debug=False)

    AW_W = MPC * NPAD + D + C_W
    d_atomw = nc.dram_tensor("atomw", [128, AW_W], F16, kind="ExternalInput")
    d_protp = [
        nc.dram_tensor(f"protp{q}", [128, 2 * L], F16, kind="ExternalInput")
        for q in range(4)
    ]
    # pnat: natural-layout prot chunks, one tensor per molecule pair
    d_pn01 = nc.dram_tensor("pn01", [128, 2 * L], F16, kind="ExternalInput")
    d_pna = nc.dram_tensor("pna", [128, 2 * L], F16, kind="ExternalInput")
    d_pn45 = nc.dram_tensor("pn45", [128, 2 * L], F16, kind="ExternalInput")
    d_pnb = nc.dram_tensor("pnb", [128, 2 * L], F16, kind="ExternalInput")
    d_cons2 = nc.dram_tensor("cons2", [128, NSTACK * D], F16, kind="ExternalInput")
    d_w1 = nc.dram_tensor("w1d", [128, 2 * H1], F16, kind="ExternalInput")
    d_w2 = nc.dram_tensor("w2d", [128, 4 * H2], F16, kind="ExternalInput")
    d_bias = nc.dram_tensor("biasc", [128, 6], F32, kind="ExternalInput")
    d_y = nc.dram_tensor("y", [MPC, 1], F32, kind="ExternalOutput")
    d_warm = nc.dram_tensor("warmo", [1, 1], F32, kind="ExternalOutput")

    with tile.TileContext(nc) as tc:
        with (
            tc.tile_pool(name="weights", bufs=1) as wpool,
            tc.tile_pool(name="work", bufs=1) as work,
            tc.tile_pool(name="psum_big", bufs=2, space=bass.MemorySpace.PSUM) as pbig,
            tc.tile_pool(name="psum_q", bufs=3, space=bass.MemorySpace.PSUM) as pq,
            tc.tile_pool(name="psum_s", bufs=3, space=bass.MemorySpace.PSUM) as ps,
        ):
            atomw = wpool.tile([128, AW_W], F16)
            protp = []
            for q in range(4):
                pt = wpool.tile([128, 2 * L], F16, tag=f"protp{q}")
                protp.append(pt)
            pn01 = wpool.tile([128, 2 * L], F16, tag="pn01")
            pna = wpool.tile([128, 2 * L], F16, tag="pna")
            pn45 = wpool.tile([128, 2 * L], F16, tag="pn45")
            pnb = wpool.tile([128, 2 * L], F16, tag="pnb")
            cons2 = wpool.tile([128, NSTACK * D], F16)
            w1 = wpool.tile([128, 2 * H1], F16, tag="w1t")
            w2 = wpool.tile([128, 4 * H2], F16, tag="w2t")
            biasc = wpool.tile([128, 6], F32)

            # HWDGE sem pool is 8: first 8 dma_start calls get fresh sems,
            # later calls reuse the earliest-completed ones (atomw/protp0).
            nc.sync.dma_start(atomw[:], d_atomw[:])          # sem 1
            nc.sync.dma_start(protp[0][:], d_protp[0][:])    # sem 2
            nc.scalar.dma_start(protp[1][:], d_protp[1][:])  # sem 3
            nc.sync.dma_start(protp[2][:], d_protp[2][:])    # sem 4
            nc.scalar.dma_start(protp[3][:], d_protp[3][:])  # sem 5
            nc.sync.dma_start(pn01[:], d_pn01[:])            # sem 6
            nc.scalar.dma_start(pna[:], d_pna[:])            # sem 7
            nc.sync.dma_start(pn45[:], d_pn45[:])            # sem 8
            nc.sync.dma_start(w1[:], d_w1[:])                # reuse 1
            nc.sync.dma_start(w2[:], d_w2[:])                # reuse 2
            nc.gpsimd.dma_start(cons2[:], d_cons2[:])
            nc.gpsimd.dma_start(pnb[:], d_pnb[:])
            nc.gpsimd.dma_start(biasc[:], d_bias[:])

            atomT = atomw[:, 0 : MPC * NPAD]
            watt = atomw[:, MPC * NPAD : MPC * NPAD + D]
            consts = atomw[:, MPC * NPAD + D :]
            protT = [protp[i // 2][:, (i % 2) * L : (i % 2 + 1) * L] for i in range(MPC)]
            pns = [pn01, pn01, pna, pna, pn45, pn45, pnb, pnb]
            pnat = [pns[i][:, (i % 2) * L : (i % 2 + 1) * L] for i in range(MPC)]
            atomN = cons2[:].rearrange("p (s d) -> p s d", s=NSTACK)

            ident = consts[:, C_IDENT : C_IDENT + 128]
            ones_col = consts[:, C_ONES : C_ONES + 1]
            ones_row = consts[0:1, C_ROW : C_ROW + 128]

            # ---- HAM warm-up: ramp the PE clock while DMAs stream ------
            warm = work.tile([128, 256], F16)
            nc.vector.memset(warm[:], 0.0)
            ps_warm = pq.tile([128, 256], F32, tag="q")
            for _ in range(N_WARM):
                nc.tensor.matmul(
                    ps_warm[:], warm[:, :128], warm[:], start=True, stop=True
                )
            warm_out = work.tile([1, 1], F32)
            nc.vector.tensor_copy(warm_out[:], ps_warm[0:1, 0:1])
            nc.gpsimd.dma_start(d_warm[:], warm_out[:])

            # ---- XT = W_att.T-applied atoms: XT[d', a] -----------------
            ps_xt = pbig.tile([128, MPC * NPAD], F32, tag="big")
            nc.tensor.matmul(ps_xt[:], watt[:], atomT[:], start=True, stop=True)
            for _ in range(N_FILL):
                nc.tensor.matmul(
                    ps_warm[:], warm[:, :128], warm[:], start=True, stop=True
                )
            xt = work.tile([128, MPC * NPAD], F16)
            nc.vector.tensor_copy(xt[:, 0:256], ps_xt[:, 0:256])
            nc.scalar.copy(xt[:, 256:512], ps_xt[:, 256:512])

            # ---- scores + transposes, pipelined per stack --------------
            s_psums = []
            s_all = work.tile([128, NSTACK * L], F16)
            ps_sts = []

            def score(s):
                ps_S = pbig.tile([128, L], F32, tag="big")
                s_psums.append(ps_S)
                for slot in range(2):
                    i = 2 * s + slot
                    nc.tensor.matmul(
                        ps_S[slot * NPAD : (slot + 1) * NPAD, :],
                        xt[:, i * NPAD : (i + 1) * NPAD],
                        protT[i],
                        start=True,
                        stop=True,
                    )
                sb = s_all[:, s * L : (s + 1) * L]
                nc.vector.tensor_copy(sb[:, 0:256], ps_S[:, 0:256])
                nc.scalar.copy(sb[:, 256:512], ps_S[:, 256:512])

            def transp(s):
                sb = s_all[:, s * L : (s + 1) * L]
                ps_st = pq.tile([128, 4 * 128], F16, tag="q")
                ps_sts.append(ps_st)
                for j in range(4):
                    nc.tensor.transpose(
                        ps_st[:, j * 128 : (j + 1) * 128],
                        sb[:, j * 128 : (j + 1) * 128],
                        ident,
                    )

            score(0)
            score(1)
            transp(0)
            score(2)
            transp(1)
            score(3)
            transp(2)
            transp(3)

            # ---- Wp / Wc per stack on vector ---------------------------
            # wpe col layout: col s = Wc; col 4+8s+2j+g = Wp(stack, chunk,
            # slot)
            wpe = work.tile([128, 4 + 8 * NSTACK], F32)
            ewx = work.tile([128, 4 + 8 * NSTACK], F16)
            wcf = work.tile([128, NSTACK], F32)
            wcf16 = work.tile([128, NSTACK], F16)

            def wp(s):
                nc.vector.reduce_max(
                    wpe[:, 4 + 8 * s : 12 + 8 * s],
                    ps_sts[s][:].rearrange("p (j g k) -> p j g k", j=4, k=NPAD),
                    axis=AxX,
                )

            def wc(s):
                nc.vector.reduce_max(
                    wpe[:, s : s + 1], s_all[:, s * L : (s + 1) * L], axis=AxX
                )

            wp(0)
            wc(0)
            wp(1)
            wc(1)
            wp(2)
            wc(2)
            wp(3)
            wc(3)

            # exps on scalar, per stack, as soon as inputs land
            def exps(s):
                nc.scalar.activation(
                    ewx[:, 4 + 8 * s : 12 + 8 * s],
                    wpe[:, 4 + 8 * s : 12 + 8 * s],
                    AF.Exp,
                )

            def expc(s):
                nc.scalar.activation(
                    wcf16[:, s : s + 1], wpe[:, s : s + 1], AF.Exp
                )

            exps(0)
            expc(0)
            exps(1)
            expc(1)
            exps(2)
            expc(2)
            exps(3)
            expc(3)

            # t partial sums (per molecule, over the 4 chunks)
            tpart = work.tile([128, MPC], F16)
            with nc.allow_low_precision(reason="sum of 4 fp16 values, 5e-4 rel"):
                nc.vector.reduce_sum(
                    tpart[:].rearrange("p (s g) -> p s g", g=2),
                    ewx[:, 4:].rearrange("p (s j g) -> p s g j", j=4, g=2),
                    axis=AxX,
                )

            # wcseg = ind * Wc (one vector op, Wc broadcast per stack pair)
            wcseg = work.tile([128, MPC], F16)
            nc.vector.tensor_mul(
                wcseg[:].rearrange("p (s o) -> p s o", o=2),
                consts[:, C_IND : C_IND + MPC].rearrange("p (s o) -> p s o", o=2),
                wcf16[:].rearrange("p (s o) -> p s o", o=1).broadcast_to((128, 4, 2)),
            )

            # ---- prot pools --------------------------------------------
            # group A (mols 0-3): row-form packed 4/column-group + transpose
            ps_prA = pq.tile([128, 128], F32, tag="q")
            for j in range(4):
                for sl in range(4):
                    m = sl
                    ewc = 4 + 8 * (m // 2) + 2 * j + (m % 2)
                    nc.tensor.matmul(
                        ps_prA[32 * sl : 32 * sl + 1, :],
                        ewx[:, ewc : ewc + 1],
                        pnat[m][:, j * 128 : (j + 1) * 128],
                        start=(j == 0),
                        stop=(j == 3),
                        tile_position=(0, 32 * sl),
                    )
            pr_sb0 = work.tile([128, 128], F16)
            nc.vector.tensor_copy(pr_sb0[:], ps_prA[:])
            ps_ppA = pq.tile([128, 128], F16, tag="q")
            nc.tensor.transpose(ps_ppA[:], pr_sb0[:], ident)

            # Sc via tiny matmuls of wcf16 against the indicator
            ps_sc = ps.tile([1, MPC], F32, tag="sp")
            for s in range(NSTACK):
                nc.tensor.matmul(
                    ps_sc[:, 2 * s : 2 * s + 2],
                    wcf16[:, s : s + 1],
                    consts[:, C_IND + 2 * s : C_IND + 2 * s + 2],
                    start=True,
                    stop=True,
                )

            # group B (mols 4-7): stationary-pnat form, columns directly
            ps_ppB = ps.tile([128, 4], F32, tag="sp")
            for mi in range(4):
                m = 4 + mi
                for j in range(4):
                    ewc = 4 + 8 * (m // 2) + 2 * j + (m % 2)
                    nc.tensor.matmul(
                        ps_ppB[:, mi : mi + 1],
                        pnat[m][:, j * 128 : (j + 1) * 128],
                        ewx[:, ewc : ewc + 1],
                        start=(j == 0),
                        stop=(j == 3),
                    )

            # ---- denominators ------------------------------------------
            ps_t = ps.tile([1, MPC], F32, tag="sp")
            nc.tensor.matmul(ps_t[:], ones_col, tpart[:], start=True, stop=True)
            sct = work.tile([1, 2 * MPC], F16)
            nc.scalar.copy(sct[0:1, :MPC], ps_sc[:])
            nc.scalar.copy(sct[0:1, MPC:], ps_t[:])
            inv = work.tile([128, 2 * MPC], F32)
            ps_bs = ps.tile([128, MPC], F32, tag="sp")
            nc.tensor.matmul(ps_bs[:], ones_row, sct[0:1, :MPC], start=True, stop=True)
            nc.vector.reciprocal(inv[:, :MPC], ps_bs[:])
            ps_bt = ps.tile([128, MPC], F32, tag="sp")
            nc.tensor.matmul(ps_bt[:], ones_row, sct[0:1, MPC:], start=True, stop=True)
            nc.vector.reciprocal(inv[:, MPC:], ps_bt[:])

            # ---- atom pool ---------------------------------------------
            ps_ap = ps.tile([128, MPC], F32, tag="sp")
            for s in range(NSTACK):
                nc.tensor.matmul(
                    ps_ap[:, 2 * s : 2 * s + 2],
                    atomN[:, s, :],
                    wcseg[:, 2 * s : 2 * s + 2],
                    start=True,
                    stop=True,
                )
            htop = work.tile([128, MPC], F16)
            nc.vector.tensor_mul(htop[:], ps_ap[:], inv[:, :MPC])
            hbot = work.tile([128, MPC], F16)
            nc.vector.tensor_mul(
                hbot[:, 0:4],
                ps_ppA[:].rearrange("p (a b) -> p b a", b=32)[:, 0, :],
                inv[:, MPC : MPC + 4],
            )
            nc.vector.tensor_mul(hbot[:, 4:8], ps_ppB[:], inv[:, MPC + 4 :])

            # ---- MLP: single group, per-chunk PSUM tiles ---------------
            h1 = work.tile([128, 4 * MPC], F16)
            for mc in range(4):
                ps_h1 = ps.tile([128, MPC], F32, tag="sp")
                nc.tensor.matmul(
                    ps_h1[:],
                    w1[:, mc * 128 : (mc + 1) * 128],
                    htop[:],
                    start=True,
                    stop=False,
                )
                nc.tensor.matmul(
                    ps_h1[:],
                    w1[:, H1 + mc * 128 : H1 + (mc + 1) * 128],
                    hbot[:],
                    start=False,
                    stop=True,
                )
                nc.scalar.activation(
                    h1[:, mc * MPC : (mc + 1) * MPC],
                    ps_h1[:],
                    AF.Relu,
                    bias=biasc[:, mc : mc + 1],
                )
            h2 = work.tile([128, 2 * MPC], F16)
            for mc2 in range(2):
                ps_h2 = ps.tile([128, MPC], F32, tag="sp")
                for kc in range(4):
                    nc.tensor.matmul(
                        ps_h2[:],
                        w2[:, kc * H2 + mc2 * 128 : kc * H2 + (mc2 + 1) * 128],
                        h1[:, kc * MPC : (kc + 1) * MPC],
                        start=(kc == 0),
                        stop=(kc == 3),
                    )
                nc.scalar.activation(
                    h2[:, mc2 * MPC : (mc2 + 1) * MPC],
                    ps_h2[:],
                    AF.Relu,
                    bias=biasc[:, 4 + mc2 : 4 + mc2 + 1],
                )
            ps_o = ps.tile([MPC, 1], F32, tag="sp")
            nc.tensor.matmul(
                ps_o[:], h2[:, :MPC], consts[:, C_WO : C_WO + 1], start=True, stop=False
            )
            nc.tensor.matmul(
                ps_o[:],
                h2[:, MPC : 2 * MPC],
                consts[:, C_WO + 1 : C_WO + 2],
                start=False,
                stop=False,
            )
            nc.tensor.matmul(
                ps_o[:],
                consts[0:1, C_ROW : C_ROW + MPC],
                consts[0:1, C_ROW + 128 : C_ROW + 129],
                start=False,
                stop=True,
            )
            y_sb = work.tile([MPC, 1], F32)
            nc.scalar.copy(y_sb[:], ps_o[:])
            nc.sync.dma_start(d_y[:], y_sb[:])

    nc.compile()
    return nc


def _prep_inputs(atom_embed, protSeq_embed, atom_splits, W_att, W1, b1, W2, b2, Wo, bo):
    f16 = np.float16
    atom = np.asarray(atom_embed, dtype=np.float32)
    prot = np.asarray(protSeq_embed, dtype=np.float32)
    splits = np.asarray(atom_splits).astype(np.int64).ravel()
    order = np.argsort(splits, kind="stable")
    counts = np.bincount(splits, minlength=B)
    assert counts.max() <= NPAD, f"molecule with {counts.max()} atoms > NPAD={NPAD}"
    assert counts.min() >= 1, "empty molecule (reference produces NaN there)"
    offs = np.concatenate([[0], np.cumsum(counts)])

    atomP = np.empty((B, NPAD, D), np.float32)
    ind = np.zeros((B, NPAD), np.float32)
    for b in range(B):
        idx = order[offs[b] : offs[b + 1]]
        n = len(idx)
        atomP[b, :n] = atom[idx]
        atomP[b, n:] = atom[idx[0]]  # replicate a real atom: maxes stay exact
        ind[b, :n] = 1.0

    w_att = np.asarray(W_att, np.float32).astype(f16)  # [128, 128]
    w1h = (
        np.asarray(W1, np.float32)
        .reshape(2, 128, H1).transpose(1, 0, 2).reshape(128, 2 * H1).astype(f16)
    )
    w2h = (
        np.asarray(W2, np.float32)
        .reshape(4, 128, H2).transpose(1, 0, 2).reshape(128, 4 * H2).astype(f16)
    )
    b1c = np.asarray(b1, np.float32).reshape(4, 128).T
    b2c = np.asarray(b2, np.float32).reshape(2, 128).T
    biasc = np.zeros((128, 6), np.float32)
    biasc[:, 0:4] = b1c
    biasc[:, 4:6] = b2c
    woc = np.asarray(Wo, np.float32).reshape(2, 128).T.astype(f16)

    in_maps = []
    for c in range(NCORES):
        sl = slice(c * MPC, (c + 1) * MPC)
        protT_c = np.ascontiguousarray(
            prot[sl].transpose(0, 2, 1).astype(f16)
        )  # [MPC, 128, L]
        pnat_c = np.ascontiguousarray(
            prot[sl].reshape(MPC, 4, 128, D).transpose(0, 2, 1, 3)
            .reshape(MPC, 128, L).astype(f16)
        )
        atomT_c = np.ascontiguousarray(atomP[sl].reshape(MPC * NPAD, D).T.astype(f16))
        atomN_c = np.ascontiguousarray(
            atomP[sl].reshape(NSTACK, 128, D).transpose(1, 0, 2)
            .reshape(128, NSTACK * D).astype(f16)
        )
        ind_c = np.zeros((128, MPC), f16)
        for m in range(MPC):
            s, slot = divmod(m, 2)
            ind_c[slot * NPAD : (slot + 1) * NPAD, m] = ind[c * MPC + m]
        consts = np.zeros((128, C_W), f16)
        consts[:, C_IDENT : C_IDENT + 128] = np.eye(128, dtype=f16)
        consts[:, C_IND : C_IND + MPC] = ind_c
        consts[:, C_ONES] = 1.0
        consts[:, C_WO : C_WO + 2] = woc
        consts[0, C_ROW : C_ROW + 128] = 1.0
        consts[0, C_ROW + 128] = np.float16(np.asarray(bo, np.float32).ravel()[0])
        im = {
            "atomw": np.ascontiguousarray(
                np.concatenate([atomT_c, w_att, consts], axis=1)
            ),
            "cons2": atomN_c,
            "w1d": w1h,
            "w2d": w2h,
            "biasc": biasc,
        }
        for q in range(4):
            im[f"protp{q}"] = np.ascontiguousarray(
                protT_c[2 * q : 2 * q + 2].transpose(1, 0, 2).reshape(128, 2 * L)
            )
        for name, mlo in (("pn01", 0), ("pna", 2), ("pn45", 4), ("pnb", 6)):
            im[name] = np.ascontiguousarray(
                pnat_c[mlo : mlo + 2].transpose(1, 0, 2).reshape(128, 2 * L)
            )
        in_maps.append(im)
    return in_maps


def kernel(atom_embed, protSeq_embed, atom_splits, W_att, W1, b1, W2, b2, Wo, bo,
           _trace=False):
    if "nc" not in _PROGRAM_CACHE:
        _PROGRAM_CACHE["nc"] = _build_program()
    nc = _PROGRAM_CACHE["nc"]
    in_maps = _prep_inputs(
        atom_embed, protSeq_embed, atom_splits, W_att, W1, b1, W2, b2, Wo, bo
    )
    res = run_bass_kernel_spmd(
        nc, in_maps, core_ids=list(range(NCORES)), trace=_trace
    )
    _PROGRAM_CACHE["last_result"] = res
    out = np.concatenate([res.results[c]["y"] for c in range(NCORES)], axis=0)
    return out.astype(np.float32)
